# revision 1
# baseline (speedup 1.0000x reference)
"""Multi-head attention (B=8, S=1024, D=768, H=12) on 8 TRN2 NeuronCores.

Sharding: batch-parallel — each core computes one batch item end-to-end
(weights replicated), so no collectives are needed. Host shards x, runs the
SPMD Bass kernel on cores 0-7, gathers per-core outputs.

Per-core dataflow (everything in transposed [feature, seq] layout so no
on-chip transposes are ever needed; matmul operands in fp16, accumulation in
fp32 PSUM):
  qT = (W_q/8)^T x^T + b_q/8      [768, 1024]  (1/sqrt(dk) folded in)
  kT = W_k^T x^T + b_k            [768, 1024]
  v  = x W_v + b_v                [1024, 768]  (natural; + ones column)
  per head h, q-block j (512 wide), k-tile i (128 wide, causal-skipped):
    scoresT = kT_h[:, i]^T qT_h[:, j]          [128, 512]  (PE, K=64)
    expT    = exp(scoresT)                     (ACT, PSUM->SBUF, fp16)
    expT   *= maskT pattern on mixed columns   (DVE)
    outU^T += [v_h | 1]^T expT                 (PE; row 64 = softmax denom)
  out^T = outU^T * bcast(1/rowsum)             (DVE; DRAM-bounce broadcast)
  yT = W_o^T out^T + b_o                       [768, 1024]
Host transposes yT back to [1024, 768].
"""

import numpy as np

B, S, D, H, DK = 8, 1024, 768, 12, 64
DT = D // 128        # 6  d-model tiles
ST = S // 128        # 8  seq tiles
NB = S // 512        # 2  512-wide seq blocks
HPM = 2              # heads per 128-row m-tile

_CACHE = {}


def _classify_mask(mask_bool):
    """mask_bool: [S, S] (q, k). Returns per (i, j) block descriptors for the
    scoresT layout [k, q]: list over j (512-wide q blocks) of lists over i
    (128-wide k tiles) of None (skip) or dict(exp_lo, mul, pat_id), plus the
    deduped mask patterns (each [128, w] float, transposed to [k, q])."""
    patterns = []
    pat_index = {}
    blocks = []
    for j in range(NB):
        row = []
        for i in range(ST):
            sub = mask_bool[j * 512:(j + 1) * 512, i * 128:(i + 1) * 128].T
            # sub: [k 128, q 512]
            if not sub.any():
                row.append(None)
                continue
            col_any = sub.any(axis=0)
            col_all = sub.all(axis=0)
            exp_lo = int(np.argmax(col_any))
            assert not col_any[:exp_lo].any()
            mixed = ~col_all
            mixed[:exp_lo] = False
            desc = {"exp_lo": exp_lo, "mul": None}
            if mixed.any():
                lo = int(np.argmax(mixed))
                hi = int(len(mixed) - np.argmax(mixed[::-1]))
                assert col_all[hi:].all() and col_all[exp_lo:lo].all()
                pat = sub[:, lo:hi].astype(np.float16)
                key = (pat.shape[1], pat.tobytes())
                if key not in pat_index:
                    pat_index[key] = len(patterns)
                    patterns.append(pat)
                desc["mul"] = (lo, hi)
                desc["pat_id"] = pat_index[key]
            row.append(desc)
        blocks.append(row)
    return blocks, patterns


def _build(blocks, patterns, pat_width):
    import concourse.bass as bass
    import concourse.bacc as bacc
    import concourse.mybir as mybir
    import concourse.tile as tile
    from contextlib import ExitStack

    f32 = mybir.dt.float32
    f16 = mybir.dt.float16
    AF = mybir.ActivationFunctionType

    nc = bacc.Bacc("TRN2", target_bir_lowering=False, debug=False)

    xT_d = nc.dram_tensor("xT", [128, DT, S], f16, kind="ExternalInput").ap()
    wq_d = nc.dram_tensor("wq", [128, DT, D], f16, kind="ExternalInput").ap()
    wk_d = nc.dram_tensor("wk", [128, DT, D], f16, kind="ExternalInput").ap()
    wv_d = nc.dram_tensor("wv", [128, DT, D], f16, kind="ExternalInput").ap()
    wo_d = nc.dram_tensor("wo", [128, DT, D], f16, kind="ExternalInput").ap()
    bq_d = nc.dram_tensor("bq", [128, DT], f32, kind="ExternalInput").ap()
    bk_d = nc.dram_tensor("bk", [128, DT], f32, kind="ExternalInput").ap()
    bv_d = nc.dram_tensor("bv", [128, H, DK], f32,
                          kind="ExternalInput").ap()
    bo_d = nc.dram_tensor("bo", [128, DT], f32, kind="ExternalInput").ap()
    yT_d = nc.dram_tensor("yT", [D, S], f32, kind="ExternalOutput").ap()
    if pat_width:
        mk_d = nc.dram_tensor("masks", [128, pat_width], f16,
                              kind="ExternalInput").ap()

    pat_off = []
    off = 0
    for p in patterns:
        pat_off.append(off)
        off += p.shape[1]

    with tile.TileContext(nc) as tc, ExitStack() as ctx:
        cpool = ctx.enter_context(tc.tile_pool(name="cpool", bufs=1))
        qT = cpool.tile([128, DT, S], f16)
        kT2 = cpool.tile([128, DT, HPM, S], f16)
        vE = cpool.tile([128, ST, H * 65 + 63], f16)
        ao = [cpool.tile([128, S], f16, name=f"ao{m}")
              for m in range(DT)]
        bvb = cpool.tile([128, H, DK], f32)
        bqs = cpool.tile([128, DT], f32)
        bks = cpool.tile([128, DT], f32)
        bos = cpool.tile([128, DT], f32)

        # big input loads: batched single DMAs on the (otherwise idle)
        # GpSimd SWDGE ring; everything else on the SP HWDGE ring
        xt = cpool.tile([128, DT, S], f16)
        wvt = cpool.tile([128, DT, D], f16)
        wqt = cpool.tile([128, DT, D], f16)
        wkt = cpool.tile([128, DT, D], f16)
        wot = cpool.tile([128, DT, D], f16)
        xT_r = xT_d
        wv_r = wv_d
        nc.sync.dma_start(out=bqs, in_=bq_d)
        nc.sync.dma_start(out=bks, in_=bk_d)
        nc.sync.dma_start(out=bos, in_=bo_d)
        nc.sync.dma_start(out=bvb, in_=bv_d)
        if pat_width:
            mks = cpool.tile([128, pat_width], f16)
            nc.sync.dma_start(out=mks, in_=mk_d)
        d_xt_last = nc.gpsimd.dma_start(out=xt, in_=xT_r)
        d_wv_last = nc.sync.dma_start(out=wvt, in_=wv_r)
        d_wq = nc.scalar.dma_start(
            out=wqt, in_=wq_d)
        d_wk = nc.gpsimd.dma_start(
            out=wkt, in_=wk_d)
        d_wo = nc.gpsimd.dma_start(
            out=wot, in_=wo_d)
        # stage the loads: x + W_v saturate the DMA engines first, then
        # W_q/W_k, then W_o — the SDMA engines round-robin all queued
        # work, so without these deps nothing finishes early
        tile.add_dep_helper(d_wq.ins, d_xt_last.ins, reason="stage inputs")
        tile.add_dep_helper(d_wk.ins, d_wv_last.ins, reason="stage inputs")
        tile.add_dep_helper(d_wo.ins, d_wq.ins, reason="stage inputs")
        tile.add_dep_helper(d_wo.ins, d_wk.ins, reason="stage inputs")
        for st in range(ST):
            ve_h = vE[:, st, 0:H * 65].rearrange("p (h e) -> p h e", e=65)
            nc.vector.memset(ve_h[:, :, DK:DK + 1], 1.0)
        nc.vector.memset(vE[:, :, H * 65:], 0.0)
        nc.vector.memset(kT2[64:128, :, 0, :], 0.0)
        nc.vector.memset(kT2[0:64, :, 1, :], 0.0)

        ps_pj = ctx.enter_context(
            tc.tile_pool(name="ps_pj", bufs=2, space="PSUM"))
        ps_s = ctx.enter_context(
            tc.tile_pool(name="ps_s", bufs=2, space="PSUM"))
        ps_o = ctx.enter_context(
            tc.tile_pool(name="ps_o", bufs=2, space="PSUM"))
        apool = ctx.enter_context(tc.tile_pool(name="apool", bufs=1))
        dpool = ctx.enter_context(
            tc.tile_pool(name="dpool", bufs=1, space="DRAM"))

        # ---- V projection (natural layout) ----
        def v_proj(h0, w):
            for st in range(ST):
                pv = ps_pj.tile([128, 512], f32, tag="pj", name="pv")
                for k in range(DT):
                    nc.tensor.matmul(
                        pv[:, :w],
                        xt[:, k, st * 128:(st + 1) * 128],
                        wvt[:, k, h0 * DK:h0 * DK + w],
                        start=(k == 0), stop=(k == DT - 1))
                nh = w // DK
                ve_h = vE[:, st, h0 * 65:(h0 + nh) * 65].rearrange(
                    "p (h e) -> p h e", e=65)
                nc.vector.tensor_add(
                    ve_h[:, :, 0:DK],
                    pv[:, :w].rearrange("p (h d) -> p h d", d=DK),
                    bvb[:, h0:h0 + nh, :])

        # ---- per m-tile: q/k projections, then attention for 2 heads ----
        rsm = None
        yas = []
        for m in range(DT):
            if m == 1:
                v_proj(8, 256)
            if m >= 4 or m % 2 == 0:
                nrow = 4 if m >= 4 else 8
                rsm = apool.tile([nrow, 512], f32, tag="rsm", bufs=2,
                                 name="rsm")
            rbase = 0 if m >= 4 else 4 * (m % 2)
            for wt, bt in ((wqt, bqs), (wkt, bks)):
                for nb in range(NB):
                    pq = ps_pj.tile([128, 512], f32, tag="pj", name="pq")
                    for k in range(DT):
                        nc.tensor.matmul(
                            pq, wt[:, k, m * 128:(m + 1) * 128],
                            xt[:, k, nb * 512:(nb + 1) * 512],
                            start=(k == 0), stop=(k == DT - 1))
                    cols = slice(nb * 512, (nb + 1) * 512)
                    if wt is wqt:
                        nc.vector.tensor_scalar_add(
                            qT[:, m, cols], pq, bt[:, m:m + 1])
                    else:
                        nc.vector.tensor_scalar_add(
                            kT2[0:64, m, 0, cols], pq[0:64, :],
                            bt[0:64, m:m + 1])
                        nc.vector.tensor_scalar_add(
                            kT2[64:128, m, 1, cols], pq[64:128, :],
                            bt[64:128, m:m + 1])

            if m == 0:
                v_proj(0, 512)
            for hh in range(HPM):
                h = m * HPM + hh
                for j in range(NB):
                    needed = [i for i in range(ST)
                              if blocks[j][i] is not None]
                    po = ps_o.tile([128, 512], f32, tag="po", name="po")
                    ets = []
                    # pair score blocks into one 2-bank PSUM tile so a
                    # single (wider) ACT exp covers both — ACT op count
                    # is the attention-phase rate limiter
                    for x in range(0, len(needed), 2):
                        # larger exp_lo in slot 0: the exp span starts at
                        # slot 0's lo, so this removes the dead gap columns
                        grp = sorted(needed[x:x + 2],
                                     key=lambda i: -blocks[j][i]["exp_lo"])
                        pss = ps_s.tile([128, 1024], f32, tag="ps",
                                        name="pss")
                        et = apool.tile([128, 1024], f16, tag="et",
                                        bufs=6, name="et")
                        offs = []
                        for t, i in enumerate(grp):
                            lo = blocks[j][i]["exp_lo"]
                            nc.tensor.matmul(
                                pss[:, t * 512 + lo:(t + 1) * 512],
                                kT2[:, m, hh, i * 128:(i + 1) * 128],
                                qT[:, m, j * 512 + lo:(j + 1) * 512],
                                start=True, stop=True)
                            offs.append((i, lo))
                        lo0 = offs[0][1]
                        hi = len(grp) * 512
                        nc.scalar.activation(out=et[:, lo0:hi],
                                             in_=pss[:, lo0:hi],
                                             func=AF.Exp)
                        for t, (i, lo) in enumerate(offs):
                            d = blocks[j][i]
                            if d["mul"] is not None:
                                mlo, mhi = d["mul"]
                                poff = pat_off[d["pat_id"]]
                                nc.vector.tensor_mul(
                                    et[:, t * 512 + mlo:t * 512 + mhi],
                                    et[:, t * 512 + mlo:t * 512 + mhi],
                                    mks[:, poff:poff + (mhi - mlo)])
                            ets.append((i, lo, et, t * 512))
                    ets.sort(key=lambda e: e[1])
                    for n, (i, lo, et, base) in enumerate(ets):
                        nc.tensor.matmul(
                            po[:, lo:512],
                            vE[:, i, h * 65:h * 65 + 128],
                            et[:, base + lo:base + 512],
                            start=(n == 0), stop=(n == len(ets) - 1))
                    # collect softmax denominator row; stash the
                    # unnormalized outU^T (normalized in-place per m-tile).
                    # PSUM is not DMA-able, so hop through SBUF via an ACT
                    # copy at the same partition, then row-DMA into rsm.
                    r = rbase + 2 * hh + j
                    tmq = apool.tile([DK + 1, 512], f32, tag="tmq", bufs=2,
                                     name="tmq")
                    nc.vector.tensor_copy(tmq[DK:DK + 1, :],
                                          po[DK:DK + 1, :])
                    nc.sync.dma_start(out=rsm[r:r + 1, :],
                                      in_=tmq[DK:DK + 1, :])
                    if hh == 0:
                        nc.vector.tensor_copy(
                            ao[m][0:DK, j * 512:(j + 1) * 512], po[0:DK, :])
                    else:
                        nc.vector.tensor_copy(
                            stg[:, j * 512:(j + 1) * 512], po[0:DK, :])
                if hh == 1:
                    nc.gpsimd.dma_start(out=ao[m][DK:128, :], in_=stg)
                else:
                    stg = apool.tile([DK, S], f16, tag="stg", bufs=2,
                                     name="stg")
            if m < 4 and m % 2 == 0:
                continue
            # batched normalization (pairs for m<4, singles for m=4,5)
            nrow = 4 if m >= 4 else 8
            rrm = apool.tile([nrow, 512], f32, tag="rrm", bufs=2, name="rrm")
            nc.vector.reciprocal(rrm, rsm[0:nrow, :])
            scr = dpool.tile([nrow, 512], f32, tag="scr", bufs=2, name="scr")
            nc.sync.dma_start(out=scr, in_=rrm)
            for mm in ((m,) if m >= 4 else (m - 1, m)):
                rt = apool.tile([128, S], f32, tag="rt", bufs=2, name="rt")
                base = scr.offset + (0 if m >= 4 else 4 * (mm % 2)) * 512
                bc0 = bass.AP(tensor=scr.tensor, offset=base,
                              ap=[[0, DK], [1, S]])
                bc1 = bass.AP(tensor=scr.tensor, offset=base + S,
                              ap=[[0, DK], [1, S]])
                nc.sync.dma_start(out=rt[0:DK, :], in_=bc0)
                nc.scalar.dma_start(out=rt[DK:128, :], in_=bc1)
                nc.vector.tensor_mul(ao[mm], ao[mm], rt)

        # ---- output projection: k=0..3 first (those ao tiles are ready
        # before the last normalize chain — keeps PE warm through it),
        # then k=4..5 with a fused add of the partial sums ----
        for m in range(DT):
            for nb in range(NB):
                g = m * NB + nb
                if g % 2 == 0:
                    py = ps_pj.tile([128, 512], f32, tag="pj", name="py")
                else:
                    pw = ps_s.tile([128, 1024], f32, tag="ps", name="pw")
                    py = pw[:, 0:512]
                for k in range(4):
                    nc.tensor.matmul(
                        py, wot[:, k, m * 128:(m + 1) * 128],
                        ao[k][:, nb * 512:(nb + 1) * 512],
                        start=(k == 0), stop=(k == 3))
                ya = apool.tile([128, 512], f32, tag="ya", bufs=12,
                                name="ya")
                nc.vector.tensor_scalar_add(ya, py, bos[:, m:m + 1])
                yas.append(ya)
        for m in range(DT):
            for nb in range(NB):
                g = m * NB + nb
                if g % 2 == 0:
                    py = ps_pj.tile([128, 512], f32, tag="pj", name="py")
                else:
                    pw = ps_s.tile([128, 1024], f32, tag="ps", name="pw")
                    py = pw[:, 0:512]
                for k in (4, 5):
                    nc.tensor.matmul(
                        py, wot[:, k, m * 128:(m + 1) * 128],
                        ao[k][:, nb * 512:(nb + 1) * 512],
                        start=(k == 4), stop=(k == 5))
                yt = apool.tile([128, 512], f32, tag="yt", bufs=3,
                                name="yt")
                nc.vector.tensor_add(yt, py, yas[m * NB + nb])
                nc.scalar.dma_start(
                    out=yT_d[m * 128:(m + 1) * 128,
                             nb * 512:(nb + 1) * 512],
                    in_=yt)

    nc.compile()
    return nc


def prepare(x, mask, W_q, b_q, W_k, b_k, W_v, b_v, W_o, b_o):
    """Compile (cached) and build per-core input maps."""
    x = np.asarray(x, np.float32)
    mask_b = np.asarray(mask).reshape(S, S) != 0
    blocks, patterns = _classify_mask(mask_b)
    key = mask_b.tobytes()
    if key not in _CACHE:
        pat_width = sum(p.shape[1] for p in patterns)
        _CACHE[key] = (_build(blocks, patterns, pat_width), patterns)
    nc, patterns = _CACHE[key]

    xT = np.ascontiguousarray(x.transpose(0, 2, 1))          # [B, D, S]

    def swz(w):
        # [D, N] -> [128, DT, N]: partition-major so each partition's DMA
        # data is one long contiguous run
        w = np.asarray(w, np.float16)
        return np.ascontiguousarray(
            w.reshape(DT, 128, w.shape[1]).transpose(1, 0, 2))

    base = {
        "wq": swz(np.asarray(W_q, np.float32) / np.sqrt(DK)),
        "wk": swz(W_k),
        "wv": swz(W_v),
        "wo": swz(W_o),
        "bq": np.ascontiguousarray(
            (np.asarray(b_q, np.float32) / np.sqrt(DK)).reshape(DT, 128).T),
        "bk": np.ascontiguousarray(
            np.asarray(b_k, np.float32).reshape(DT, 128).T),
        "bv": np.ascontiguousarray(np.broadcast_to(
            np.asarray(b_v, np.float32).reshape(1, H, DK), (128, H, DK))),
        "bo": np.ascontiguousarray(
            np.asarray(b_o, np.float32).reshape(DT, 128).T),
    }
    if patterns:
        base["masks"] = np.ascontiguousarray(np.concatenate(patterns, axis=1))
    in_maps = [dict(base, xT=swz(xT[c])) for c in range(B)]
    return nc, in_maps


def kernel(**inputs):
    from concourse.bass_utils import run_bass_kernel_spmd

    nc, in_maps = prepare(**inputs)
    res = run_bass_kernel_spmd(nc, in_maps, core_ids=list(range(B)))
    out = np.stack([res.results[c]["yT"].T for c in range(B)], axis=0)
    return np.ascontiguousarray(out.astype(np.float32))



# revision 5
# speedup vs baseline: 1.0025x; 1.0025x over previous
"""Multi-head attention (B=8, S=1024, D=768, H=12) on 8 TRN2 NeuronCores.

Sharding: batch-parallel — each core computes one batch item end-to-end
(weights replicated), so no collectives are needed. Host shards x, runs the
SPMD Bass kernel on cores 0-7, gathers per-core outputs.

Per-core dataflow ([feature, seq] transposed layout, fp16 matmuls, fp32 PSUM):
  qT = (W_q/8)^T x^T          [768, 1024]  partitions 0-63 = head 2m,
  kT = W_k^T x^T              [768, 1024]  64-127 = head 2m+1 per m-tile
  v  = x W_v (+ ones col)     [1024, 768]
  scores: per (m, j-block, k-tile i): TWO K=64 matmuls (head pair) into one
    PSUM tile — auto row-tiling (tile_position (0,0)/(64,0)) runs them
    concurrently in the 64x128-tiled PE array.
  exp: single gap-free ACT op per group covering both heads (ACT is the
    attention-phase bottleneck: (N+352)/1.2ns per op).
  attnV: per head, M=65 (64 v-dims + ones row -> softmax denom), N trimmed
    causally; evacuated as one [65,512] DVE copy, rows fanned out by DMA.
  normalize per m: denom rows -> DRAM, reload as [64,32] (all lanes),
    reciprocal, DRAM-bounce broadcast, one [128,1024] fp16 multiply.
  yT = W_o^T out^T: pass1 (k=0..3) interleaved into m=4 attention stalls,
    pass2 (k=4..5) after the last normalize; fp16 stores.
Host transposes/upcasts yT back to [1024, 768] fp32.
"""

import numpy as np

B, S, D, H, DK = 8, 1024, 768, 12, 64
DT = D // 128        # 6  d-model tiles
ST = S // 128        # 8  seq tiles
NB = S // 512        # 2  512-wide seq blocks
HPM = 2              # heads per 128-row m-tile

_CACHE = {}


def _classify_mask(mask_bool):
    """mask_bool: [S, S] (q, k). Per (j, i) block descriptors for the
    scoresT layout [k, q]: list over j (512-wide q blocks) of lists of
    (i, exp_lo, mul(lo,hi)|None, pat_id|None), plus deduped mask patterns
    (each [128, w] f16, [k, q], stored duplicated side by side so one DVE
    op can mask both heads' copies)."""
    patterns = []
    pat_index = {}
    blocks = []
    for j in range(NB):
        row = []
        for i in range(ST):
            sub = mask_bool[j * 512:(j + 1) * 512, i * 128:(i + 1) * 128].T
            # sub: [k 128, q 512]
            if not sub.any():
                continue
            col_any = sub.any(axis=0)
            col_all = sub.all(axis=0)
            exp_lo = int(np.argmax(col_any))
            assert not col_any[:exp_lo].any()
            mixed = ~col_all
            mixed[:exp_lo] = False
            mul = None
            pat_id = None
            if mixed.any():
                lo = int(np.argmax(mixed))
                hi = int(len(mixed) - np.argmax(mixed[::-1]))
                assert col_all[hi:].all() and col_all[exp_lo:lo].all()
                pat = sub[:, lo:hi].astype(np.float16)
                key = (pat.shape[1], pat.tobytes())
                if key not in pat_index:
                    pat_index[key] = len(patterns)
                    patterns.append(pat)
                mul = (lo, hi)
                pat_id = pat_index[key]
            row.append((i, exp_lo, mul, pat_id))
        blocks.append(row)
    return blocks, patterns


def _build(blocks, patterns, pat_width):
    import concourse.bass as bass
    import concourse.bacc as bacc
    import concourse.mybir as mybir
    import concourse.tile as tile
    from contextlib import ExitStack

    f32 = mybir.dt.float32
    f16 = mybir.dt.float16
    AF = mybir.ActivationFunctionType

    nc = bacc.Bacc("TRN2", target_bir_lowering=False, debug=False)

    xT_d = nc.dram_tensor("xT", [128, DT, S], f16, kind="ExternalInput").ap()
    wq_d = nc.dram_tensor("wq", [128, DT, DT, 128], f16,
                          kind="ExternalInput").ap()   # [p, m, k, c]
    wk_d = nc.dram_tensor("wk", [128, DT, DT, 128], f16,
                          kind="ExternalInput").ap()
    wv_d = nc.dram_tensor("wv", [128, DT, D], f16, kind="ExternalInput").ap()
    wo_d = nc.dram_tensor("wo", [128, DT, D], f16, kind="ExternalInput").ap()
    bq_d = nc.dram_tensor("bq", [128, DT], f32, kind="ExternalInput").ap()
    bk_d = nc.dram_tensor("bk", [128, DT], f32, kind="ExternalInput").ap()
    bv_d = nc.dram_tensor("bv", [128, H, DK], f32,
                          kind="ExternalInput").ap()
    bo_d = nc.dram_tensor("bo", [128, DT], f32, kind="ExternalInput").ap()
    yT_d = nc.dram_tensor("yT", [D, S], f16, kind="ExternalOutput").ap()
    if pat_width:
        mk_d = nc.dram_tensor("masks", [128, pat_width], f16,
                              kind="ExternalInput").ap()

    pat_off = []
    off = 0
    for p in patterns:
        pat_off.append(off)
        off += 2 * p.shape[1]

    with tile.TileContext(nc) as tc, ExitStack() as ctx:
        cpool = ctx.enter_context(tc.tile_pool(name="cpool", bufs=1))
        qT = cpool.tile([128, DT, S], f16)
        kT = cpool.tile([128, DT, S], f16)
        vE = cpool.tile([128, ST, H * 65 + 63], f16)
        ao = [cpool.tile([128, S], f16, name=f"ao{m}") for m in range(DT)]
        ya = [cpool.tile([128, 512], f16, name=f"ya{g}") for g in range(12)]
        bvb = cpool.tile([128, H, DK], f32)
        bqs = cpool.tile([128, DT], f32)
        bks = cpool.tile([128, DT], f32)
        bos = cpool.tile([128, DT], f32)

        xt = cpool.tile([128, DT, S], f16)
        wvt = cpool.tile([128, DT, D], f16)
        wqt = cpool.tile([128, DT, DT, 128], f16)
        wkt = cpool.tile([128, DT, DT, 128], f16)
        wot = cpool.tile([128, DT, D], f16)

        # small constants (fire immediately, no chain)
        nc.sync.dma_start(out=bqs, in_=bq_d)
        nc.sync.dma_start(out=bks, in_=bk_d)
        nc.sync.dma_start(out=bos, in_=bo_d)
        nc.sync.dma_start(out=bvb, in_=bv_d)
        if pat_width:
            mks = cpool.tile([128, pat_width], f16)
            nc.sync.dma_start(out=mks, in_=mk_d)

        # staged input loads. x + first Wq/Wk m-tiles gate the first
        # matmuls; chain the rest so the SDMA round-robin doesn't starve
        # the early pieces.
        d_x0 = nc.gpsimd.dma_start(out=xt[:, 0:3], in_=xT_d[:, 0:3])
        d_x1 = nc.gpsimd.dma_start(out=xt[:, 3:6], in_=xT_d[:, 3:6])
        d_wva = nc.gpsimd.dma_start(out=wvt[:, :, 0:512],
                                    in_=wv_d[:, :, 0:512])
        d_wvb = nc.gpsimd.dma_start(out=wvt[:, :, 512:768],
                                    in_=wv_d[:, :, 512:768])
        d_wo = nc.gpsimd.dma_start(out=wot, in_=wo_d)
        d_wq0 = nc.sync.dma_start(out=wqt[:, 0], in_=wq_d[:, 0])
        d_wk0 = nc.sync.dma_start(out=wkt[:, 0], in_=wk_d[:, 0])
        d_wq1 = nc.sync.dma_start(out=wqt[:, 1:6], in_=wq_d[:, 1:6])
        d_wk1 = nc.sync.dma_start(out=wkt[:, 1:6], in_=wk_d[:, 1:6])
        for a, b in ((d_x1, d_x0), (d_wva, d_x1), (d_wvb, d_wva),
                     (d_wo, d_wvb), (d_wk0, d_wq0), (d_wq1, d_wk0),
                     (d_wk1, d_wq1)):
            tile.add_dep_helper(a.ins, b.ins, reason="stage inputs")

        # ones columns for the softmax-denominator rows; zero the tail so
        # the last heads' 128-col lhsT windows read defined data
        for st in range(ST):
            ve_h = vE[:, st, 0:H * 65].rearrange("p (h e) -> p h e", e=65)
            nc.vector.memset(ve_h[:, :, DK:DK + 1], 1.0)
        nc.vector.memset(vE[:, :, H * 65:], 0.0)

        ps_pj = ctx.enter_context(
            tc.tile_pool(name="ps_pj", bufs=2, space="PSUM"))
        ps_s = ctx.enter_context(
            tc.tile_pool(name="ps_s", bufs=2, space="PSUM"))
        ps_o = ctx.enter_context(
            tc.tile_pool(name="ps_o", bufs=2, space="PSUM"))
        apool = ctx.enter_context(tc.tile_pool(name="apool", bufs=1))
        dpool = ctx.enter_context(
            tc.tile_pool(name="dpool", bufs=1, space="DRAM"))

        def q_proj(m):
            for nb in range(NB):
                pq = ps_pj.tile([128, 512], f32, tag="pj", name="pq")
                for k in range(DT):
                    nc.tensor.matmul(
                        pq, wqt[:, m, k], xt[:, k, nb * 512:(nb + 1) * 512],
                        start=(k == 0), stop=(k == DT - 1))
                nc.vector.tensor_scalar_add(
                    qT[:, m, nb * 512:(nb + 1) * 512], pq, bqs[:, m:m + 1])

        def k_proj(m):
            for nb in range(NB):
                pk = ps_pj.tile([128, 512], f32, tag="pj", name="pk")
                for k in range(DT):
                    nc.tensor.matmul(
                        pk, wkt[:, m, k], xt[:, k, nb * 512:(nb + 1) * 512],
                        start=(k == 0), stop=(k == DT - 1))
                nc.vector.tensor_scalar_add(
                    kT[:, m, nb * 512:(nb + 1) * 512], pk, bks[:, m:m + 1])

        def v_proj(h0, w, sts):
            for st in sts:
                pv = ps_pj.tile([128, 512], f32, tag="pj", name="pv")
                for k in range(DT):
                    nc.tensor.matmul(
                        pv[:, :w],
                        xt[:, k, st * 128:(st + 1) * 128],
                        wvt[:, k, h0 * DK:h0 * DK + w],
                        start=(k == 0), stop=(k == DT - 1))
                nh = w // DK
                ve_h = vE[:, st, h0 * 65:(h0 + nh) * 65].rearrange(
                    "p (h e) -> p h e", e=65)
                nc.vector.tensor_add(
                    ve_h[:, :, 0:DK],
                    pv[:, :w].rearrange("p (h d) -> p h d", d=DK),
                    bvb[:, h0:h0 + nh, :])

        def oproj_pass(g, ks, first):
            mo, nb = g // NB, g % NB
            py = ps_pj.tile([128, 512], f32, tag="pj", name="py")
            for n, k in enumerate(ks):
                nc.tensor.matmul(
                    py, wot[:, k, mo * 128:(mo + 1) * 128],
                    ao[k][:, nb * 512:(nb + 1) * 512],
                    start=(n == 0), stop=(n == len(ks) - 1))
            if first:
                nc.vector.tensor_scalar_add(ya[g], py, bos[:, mo:mo + 1])
            else:
                yt = apool.tile([128, 512], f16, tag="yt", bufs=3, name="yt")
                nc.vector.tensor_add(yt, py, ya[g])
                nc.gpsimd.dma_start(
                    out=yT_d[mo * 128:(mo + 1) * 128,
                             nb * 512:(nb + 1) * 512],
                    in_=yt)

        def scores(m, j):
            """Emit paired-head score matmuls + exp + mask for all groups
            of (m, j). Returns [(i, lo, w, et)] for attnV."""
            out = []
            for (i, lo, mul, pat_id) in blocks[j]:
                w = 512 - lo
                pss = ps_s.tile([128, 1024], f32, tag="ps", name="pss")
                et = apool.tile([128, 1024], f16, tag="et", bufs=10,
                                name="et")
                for hh in range(HPM):
                    p0, p1 = hh * 64, hh * 64 + 64
                    nc.tensor.matmul(
                        pss[:, hh * 512 + lo:(hh + 1) * 512],
                        kT[p0:p1, m, i * 128:(i + 1) * 128],
                        qT[p0:p1, m, j * 512 + lo:(j + 1) * 512],
                        start=True, stop=True)
                # one ACT op for both heads; strided AP skips the dead
                # [512-lo, 512) gap (matmul outs can't cross PSUM banks)
                p3 = pss.rearrange("p (two q) -> p two q", two=2)
                e3 = et.rearrange("p (two q) -> p two q", two=2)
                nc.scalar.activation(out=e3[:, :, lo:512],
                                     in_=p3[:, :, lo:512], func=AF.Exp)
                if mul is not None:
                    mlo, mhi = mul
                    wm = mhi - mlo
                    m3 = mks[:, pat_off[pat_id]:pat_off[pat_id] + 2 * wm
                             ].rearrange("p (two q) -> p two q", two=2)
                    nc.vector.tensor_mul(
                        e3[:, :, mlo:mhi], e3[:, :, mlo:mhi], m3)
                out.append((i, lo, w, et))
            return out

        def attn_v(m, j, ets, drow):
            for hh in range(HPM):
                h = m * HPM + hh
                po = ps_o.tile([128, 512], f32, tag="po", name="po")
                for n, (i, lo, w, et) in enumerate(ets):
                    nc.tensor.matmul(
                        po[:, lo:512],
                        vE[:, i, h * 65:h * 65 + 128],
                        et[:, hh * 512 + lo:(hh + 1) * 512],
                        start=(n == 0), stop=(n == len(ets) - 1))
                stg = apool.tile([65, 512], f16, tag="stg", bufs=4,
                                 name="stg")
                nc.vector.tensor_copy(stg, po[0:65, :])
                nc.gpsimd.dma_start(
                    out=ao[m][hh * DK:(hh + 1) * DK,
                              j * 512:(j + 1) * 512],
                    in_=stg[0:DK, :])
                nc.sync.dma_start(out=drow[hh * NB + j:hh * NB + j + 1, :],
                                  in_=stg[DK:65, :])

        def normalize(m, drow):
            rsm = apool.tile([64, 32], f16, tag="rsm", bufs=2, name="rsm")
            nc.sync.dma_start(
                out=rsm, in_=drow.rearrange("a (b c) -> (a b) c", b=16))
            rrm = apool.tile([64, 32], f32, tag="rrm", bufs=2, name="rrm")
            nc.vector.reciprocal(rrm, rsm)
            rc16 = apool.tile([64, 32], f16, tag="rc16", bufs=2, name="rc16")
            nc.vector.tensor_copy(rc16, rrm)
            scr = dpool.tile([64, 32], f16, tag="scr", bufs=2, name="scr")
            nc.sync.dma_start(out=scr, in_=rc16)
            rt = apool.tile([128, S], f16, tag="rt", bufs=2, name="rt")
            bc0 = bass.AP(tensor=scr.tensor, offset=scr.offset,
                          ap=[[0, DK], [1, S]])
            bc1 = bass.AP(tensor=scr.tensor, offset=scr.offset + S,
                          ap=[[0, DK], [1, S]])
            nc.sync.dma_start(out=rt[0:DK, :], in_=bc0)
            nc.gpsimd.dma_start(out=rt[DK:128, :], in_=bc1)
            nc.vector.tensor_mul(ao[m], ao[m], rt)

        # ---- main pipeline over m-tiles ----
        q_proj(0)
        k_proj(0)
        v_proj(0, 512, range(0, 4))
        for m in range(DT):
            drow = dpool.tile([HPM * NB, 512], f16, tag="drow", bufs=2,
                              name="drow")
            ets0 = scores(m, 0)
            # FILL_A: 128-mode PE work to hide exp(j0) on ACT
            if m == 0:
                v_proj(0, 512, range(4, 8))
            elif m == 1:
                v_proj(8, 256, range(0, 8))
            elif m < 5:
                q_proj(m + 1)
            if m == 4:
                for g in range(0, 4):
                    oproj_pass(g, range(4), True)
            attn_v(m, 0, ets0, drow)
            ets1 = scores(m, 1)
            # FILL_B: hide exp(j1)
            if m == 0:
                q_proj(1)
                k_proj(1)
            elif m == 1:
                k_proj(2)
                q_proj(2)
            elif m < 5:
                k_proj(m + 1)
            if m == 4:
                for g in range(4, 12):
                    oproj_pass(g, range(4), True)
            attn_v(m, 1, ets1, drow)
            normalize(m, drow)
        for g in range(12):
            oproj_pass(g, (4, 5), False)

    nc.compile()
    return nc


def prepare(x, mask, W_q, b_q, W_k, b_k, W_v, b_v, W_o, b_o):
    """Compile (cached) and build per-core input maps."""
    x = np.asarray(x, np.float32)
    mask_b = np.asarray(mask).reshape(S, S) != 0
    blocks, patterns = _classify_mask(mask_b)
    key = mask_b.tobytes()
    if key not in _CACHE:
        pat_width = sum(2 * p.shape[1] for p in patterns)
        _CACHE[key] = (_build(blocks, patterns, pat_width), patterns)
    nc, patterns = _CACHE[key]

    xT = np.ascontiguousarray(x.transpose(0, 2, 1))          # [B, D, S]

    def swz(w):
        # [D, N] -> [128, DT, N]: partition-major
        w = np.asarray(w, np.float16)
        return np.ascontiguousarray(
            w.reshape(DT, 128, w.shape[1]).transpose(1, 0, 2))

    def swz_m(w):
        # [D, D] -> [128, m, k, 128] so per-m slices are contiguous
        w = np.asarray(w, np.float16)
        return np.ascontiguousarray(
            w.reshape(DT, 128, DT, 128).transpose(1, 2, 0, 3))

    base = {
        "wq": swz_m(np.asarray(W_q, np.float32) / np.sqrt(DK)),
        "wk": swz_m(W_k),
        "wv": swz(W_v),
        "wo": swz(W_o),
        "bq": np.ascontiguousarray(
            (np.asarray(b_q, np.float32) / np.sqrt(DK)).reshape(DT, 128).T),
        "bk": np.ascontiguousarray(
            np.asarray(b_k, np.float32).reshape(DT, 128).T),
        "bv": np.ascontiguousarray(np.broadcast_to(
            np.asarray(b_v, np.float32).reshape(1, H, DK), (128, H, DK))),
        "bo": np.ascontiguousarray(
            np.asarray(b_o, np.float32).reshape(DT, 128).T),
    }
    if patterns:
        base["masks"] = np.ascontiguousarray(
            np.concatenate([np.concatenate([p, p], axis=1)
                            for p in patterns], axis=1))
    in_maps = [dict(base, xT=swz(xT[c])) for c in range(B)]
    return nc, in_maps


def kernel(**inputs):
    from concourse.bass_utils import run_bass_kernel_spmd

    nc, in_maps = prepare(**inputs)
    res = run_bass_kernel_spmd(nc, in_maps, core_ids=list(range(B)))
    out = np.stack([res.results[c]["yT"].T.astype(np.float32)
                    for c in range(B)], axis=0)
    return np.ascontiguousarray(out)


# revision 10
# speedup vs baseline: 1.0753x; 1.0726x over previous
"""Multi-head attention (B=8, S=1024, D=768, H=12) on 8 TRN2 NeuronCores.

Sharding: batch-parallel — each core computes one batch item end-to-end
(weights replicated), so no collectives are needed. Host shards x, runs the
SPMD Bass kernel on cores 0-7, gathers per-core outputs.

Per-core dataflow ([feature, seq] transposed layout, fp16 matmuls, fp32 PSUM):
  qT = (W_q/8)^T x^T          [768, 1024]  partitions 0-63 = head 2m,
  kT = W_k^T x^T              [768, 1024]  64-127 = head 2m+1 per m-tile
  v  = x W_v (+ ones col)     [1024, 768]
  scores: per (m, j-block, k-tile i): TWO K=64 matmuls (head pair) into one
    PSUM tile — auto row-tiling (tile_position (0,0)/(64,0)) runs them
    concurrently in the 64x128-tiled PE array.
  exp: single gap-free ACT op per group covering both heads (ACT is the
    attention-phase bottleneck: (N+352)/1.2ns per op).
  attnV: per head, M=65 (64 v-dims + ones row -> softmax denom), N trimmed
    causally; evacuated as one [65,512] DVE copy, rows fanned out by DMA.
  normalize per m: denom rows -> DRAM, reload as [64,32] (all lanes),
    reciprocal, DRAM-bounce broadcast, one [128,1024] fp16 multiply.
  yT = W_o^T out^T: pass1 (k=0..3) interleaved into m=4 attention stalls,
    pass2 (k=4..5) after the last normalize; fp16 stores.
Host transposes/upcasts yT back to [1024, 768] fp32.
"""

import numpy as np

B, S, D, H, DK = 8, 1024, 768, 12, 64
DT = D // 128        # 6  d-model tiles
ST = S // 128        # 8  seq tiles
NB = S // 512        # 2  512-wide seq blocks
HPM = 2              # heads per 128-row m-tile

_CACHE = {}


def _classify_mask(mask_bool):
    """mask_bool: [S, S] (q, k). Per (j, i) block descriptors for the
    scoresT layout [k, q]: list over j (512-wide q blocks) of lists of
    (i, exp_lo, mul(lo,hi)|None, pat_id|None), plus deduped mask patterns
    (each [128, w] f16, [k, q], stored duplicated side by side so one DVE
    op can mask both heads' copies)."""
    patterns = []
    pat_index = {}
    blocks = []
    for j in range(NB):
        row = []
        for i in range(ST):
            sub = mask_bool[j * 512:(j + 1) * 512, i * 128:(i + 1) * 128].T
            # sub: [k 128, q 512]
            if not sub.any():
                continue
            col_any = sub.any(axis=0)
            col_all = sub.all(axis=0)
            exp_lo = int(np.argmax(col_any))
            assert not col_any[:exp_lo].any()
            mixed = ~col_all
            mixed[:exp_lo] = False
            mul = None
            pat_id = None
            if mixed.any():
                lo = int(np.argmax(mixed))
                hi = int(len(mixed) - np.argmax(mixed[::-1]))
                assert col_all[hi:].all() and col_all[exp_lo:lo].all()
                pat = sub[:, lo:hi].astype(np.float16)
                key = (pat.shape[1], pat.tobytes())
                if key not in pat_index:
                    pat_index[key] = len(patterns)
                    patterns.append(pat)
                mul = (lo, hi)
                pat_id = pat_index[key]
            row.append((i, exp_lo, mul, pat_id))
        blocks.append(row)
    return blocks, patterns


def _build(blocks, patterns, pat_width):
    import concourse.bass as bass
    import concourse.bacc as bacc
    import concourse.mybir as mybir
    import concourse.tile as tile
    from contextlib import ExitStack

    f32 = mybir.dt.float32
    f16 = mybir.dt.float16
    AF = mybir.ActivationFunctionType

    nc = bacc.Bacc("TRN2", target_bir_lowering=False, debug=False)

    xT_d = nc.dram_tensor("xT", [128, DT, S], f16, kind="ExternalInput").ap()
    wq_d = nc.dram_tensor("wq", [128, DT, DT, 128], f16,
                          kind="ExternalInput").ap()   # [p, m, k, c]
    wk_d = nc.dram_tensor("wk", [128, DT, DT, 128], f16,
                          kind="ExternalInput").ap()
    wv_d = nc.dram_tensor("wv", [128, DT, D], f16, kind="ExternalInput").ap()
    wo_d = nc.dram_tensor("wo", [128, DT, D], f16, kind="ExternalInput").ap()
    bq_d = nc.dram_tensor("bq", [128, DT], f32, kind="ExternalInput").ap()
    bk_d = nc.dram_tensor("bk", [128, DT], f32, kind="ExternalInput").ap()
    bv_d = nc.dram_tensor("bv", [128, H, DK], f32,
                          kind="ExternalInput").ap()
    bo_d = nc.dram_tensor("bo", [128, DT], f32, kind="ExternalInput").ap()
    yT_d = nc.dram_tensor("yT", [D, S], f16, kind="ExternalOutput").ap()
    if pat_width:
        mk_d = nc.dram_tensor("masks", [128, pat_width], f16,
                              kind="ExternalInput").ap()

    pat_off = []
    off = 0
    for p in patterns:
        pat_off.append(off)
        off += 2 * p.shape[1]

    with tile.TileContext(nc) as tc, ExitStack() as ctx:
        cpool = ctx.enter_context(tc.tile_pool(name="cpool", bufs=1))
        qT = cpool.tile([128, DT, S], f16)
        kT = cpool.tile([128, DT, S], f16)
        vE = cpool.tile([128, ST, H * 65 + 63], f16)
        ao = [cpool.tile([128, S], f16, name=f"ao{m}") for m in range(DT)]
        ya = [cpool.tile([128, 512], f16, name=f"ya{g}") for g in range(12)]
        bvb = cpool.tile([128, H, DK], f32)
        bqs = cpool.tile([128, DT], f32)
        bks = cpool.tile([128, DT], f32)
        bos = cpool.tile([128, DT], f32)

        xt = cpool.tile([128, DT, S], f16)
        wvt = cpool.tile([128, DT, D], f16)
        wqt = cpool.tile([128, DT, DT, 128], f16)
        wkt = cpool.tile([128, DT, DT, 128], f16)
        wot = cpool.tile([128, DT, D], f16)

        # staged input loads across both DMA rings. x chunks + wq0/wk0
        # gate the first matmuls; chains keep the SDMA round-robin from
        # starving the early pieces.
        mks = cpool.tile([128, pat_width], f16, name="mks") \
            if pat_width else None
        d_x0 = nc.gpsimd.dma_start(out=xt[:, 0:2], in_=xT_d[:, 0:2])
        d_x1 = nc.gpsimd.dma_start(out=xt[:, 2:4], in_=xT_d[:, 2:4])
        d_wva = nc.gpsimd.dma_start(out=wvt[:, :, 0:512],
                                    in_=wv_d[:, :, 0:512])
        d_wvb = nc.gpsimd.dma_start(out=wvt[:, :, 512:768],
                                    in_=wv_d[:, :, 512:768])
        d_wo = nc.gpsimd.dma_start(out=wot, in_=wo_d)
        gchain = [d_x0, d_x1, d_wva, d_wvb, d_wo]
        d_x2 = nc.sync.dma_start(out=xt[:, 4:6], in_=xT_d[:, 4:6])
        d_wq0 = nc.sync.dma_start(out=wqt[:, 0], in_=wq_d[:, 0])
        d_wk0 = nc.sync.dma_start(out=wkt[:, 0], in_=wk_d[:, 0])
        d_bq = nc.sync.dma_start(out=bqs, in_=bq_d)
        d_bk = nc.sync.dma_start(out=bks, in_=bk_d)
        schain = [d_x2, d_wq0, d_wk0, d_bq, d_bk]
        if pat_width:
            schain.append(nc.sync.dma_start(out=mks, in_=mk_d))
        schain.append(nc.sync.dma_start(out=bvb, in_=bv_d))
        schain.append(nc.sync.dma_start(out=wqt[:, 1:6], in_=wq_d[:, 1:6]))
        schain.append(nc.sync.dma_start(out=wkt[:, 1:6], in_=wk_d[:, 1:6]))
        schain.append(nc.sync.dma_start(out=bos, in_=bo_d))
        for chain in (gchain, schain):
            for a, b in zip(chain[1:], chain):
                tile.add_dep_helper(a.ins, b.ins, reason="stage inputs")

        # ones columns for the softmax-denominator rows; zero the tail so
        # the last heads' 128-col lhsT windows read defined data
        for st in range(ST):
            ve_h = vE[:, st, 0:H * 65].rearrange("p (h e) -> p h e", e=65)
            nc.vector.memset(ve_h[:, :, DK:DK + 1], 1.0)
        nc.vector.memset(vE[:, :, H * 65:], 0.0)

        ps_pj = ctx.enter_context(
            tc.tile_pool(name="ps_pj", bufs=2, space="PSUM"))
        ps_s = ctx.enter_context(
            tc.tile_pool(name="ps_s", bufs=2, space="PSUM"))
        ps_o = ctx.enter_context(
            tc.tile_pool(name="ps_o", bufs=2, space="PSUM"))
        apool = ctx.enter_context(tc.tile_pool(name="apool", bufs=1))
        dpool = ctx.enter_context(
            tc.tile_pool(name="dpool", bufs=1, space="DRAM"))

        def q_proj(m):
            for nb in range(NB):
                pq = ps_pj.tile([128, 512], f32, tag="pj", name="pq")
                for k in range(DT):
                    nc.tensor.matmul(
                        pq, wqt[:, m, k], xt[:, k, nb * 512:(nb + 1) * 512],
                        start=(k == 0), stop=(k == DT - 1))
                nc.vector.tensor_scalar_add(
                    qT[:, m, nb * 512:(nb + 1) * 512], pq, bqs[:, m:m + 1])

        def k_proj(m):
            for nb in range(NB):
                pk = ps_pj.tile([128, 512], f32, tag="pj", name="pk")
                for k in range(DT):
                    nc.tensor.matmul(
                        pk, wkt[:, m, k], xt[:, k, nb * 512:(nb + 1) * 512],
                        start=(k == 0), stop=(k == DT - 1))
                nc.vector.tensor_scalar_add(
                    kT[:, m, nb * 512:(nb + 1) * 512], pk, bks[:, m:m + 1])

        def v_proj(h0, w, sts):
            for st in sts:
                pv = ps_pj.tile([128, 512], f32, tag="pj", name="pv")
                for k in range(DT):
                    nc.tensor.matmul(
                        pv[:, :w],
                        xt[:, k, st * 128:(st + 1) * 128],
                        wvt[:, k, h0 * DK:h0 * DK + w],
                        start=(k == 0), stop=(k == DT - 1))
                nh = w // DK
                ve_h = vE[:, st, h0 * 65:(h0 + nh) * 65].rearrange(
                    "p (h e) -> p h e", e=65)
                nc.vector.tensor_add(
                    ve_h[:, :, 0:DK],
                    pv[:, :w].rearrange("p (h d) -> p h d", d=DK),
                    bvb[:, h0:h0 + nh, :])

        def oproj_pass(g, ks, first):
            mo, nb = g // NB, g % NB
            py = ps_pj.tile([128, 512], f32, tag="pj", name="py")
            for n, k in enumerate(ks):
                nc.tensor.matmul(
                    py, wot[:, k, mo * 128:(mo + 1) * 128],
                    ao[k][:, nb * 512:(nb + 1) * 512],
                    start=(n == 0), stop=(n == len(ks) - 1))
            if first:
                nc.vector.tensor_scalar_add(ya[g], py, bos[:, mo:mo + 1])
            else:
                yt = apool.tile([128, 512], f16, tag="yt", bufs=3, name="yt")
                nc.vector.tensor_add(yt, py, ya[g])
                eng = nc.sync if g % 2 == 0 else nc.gpsimd
                eng.dma_start(
                    out=yT_d[mo * 128:(mo + 1) * 128,
                             nb * 512:(nb + 1) * 512],
                    in_=yt)

        def scores(m, j):
            """Emit paired-head score matmuls + exp + mask for all groups
            of (m, j). Returns [(i, lo, w, et)] for attnV."""
            out = []
            for (i, lo, mul, pat_id) in blocks[j]:
                w = 512 - lo
                pss = ps_s.tile([128, 1024], f32, tag="ps", name="pss")
                et = apool.tile([128, 1024], f16, tag="et", bufs=10,
                                name="et")
                for hh in range(HPM):
                    p0, p1 = hh * 64, hh * 64 + 64
                    nc.tensor.matmul(
                        pss[:, hh * 512 + lo:(hh + 1) * 512],
                        kT[p0:p1, m, i * 128:(i + 1) * 128],
                        qT[p0:p1, m, j * 512 + lo:(j + 1) * 512],
                        start=True, stop=True)
                # one ACT op for both heads; strided AP skips the dead
                # [512-lo, 512) gap (matmul outs can't cross PSUM banks)
                p3 = pss.rearrange("p (two q) -> p two q", two=2)
                e3 = et.rearrange("p (two q) -> p two q", two=2)
                nc.scalar.activation(out=e3[:, :, lo:512],
                                     in_=p3[:, :, lo:512], func=AF.Exp)
                if mul is not None:
                    mlo, mhi = mul
                    wm = mhi - mlo
                    m3 = mks[:, pat_off[pat_id]:pat_off[pat_id] + 2 * wm
                             ].rearrange("p (two q) -> p two q", two=2)
                    nc.vector.tensor_mul(
                        e3[:, :, mlo:mhi], e3[:, :, mlo:mhi], m3)
                out.append((i, lo, w, et))
            return out

        def attn_v(m, j, ets, drow):
            for hh in range(HPM):
                h = m * HPM + hh
                po = ps_o.tile([128, 512], f32, tag="po", name="po")
                for n, (i, lo, w, et) in enumerate(ets):
                    nc.tensor.matmul(
                        po[:, lo:512],
                        vE[:, i, h * 65:h * 65 + 128],
                        et[:, hh * 512 + lo:(hh + 1) * 512],
                        start=(n == 0), stop=(n == len(ets) - 1))
                stg = apool.tile([65, 512], f16, tag="stg", bufs=4,
                                 name="stg")
                nc.vector.tensor_copy(stg, po[0:65, :])
                nc.gpsimd.dma_start(
                    out=ao[m][hh * DK:(hh + 1) * DK,
                              j * 512:(j + 1) * 512],
                    in_=stg[0:DK, :])
                nc.sync.dma_start(out=drow[hh * NB + j:hh * NB + j + 1, :],
                                  in_=stg[DK:65, :])

        def normalize(m, drow):
            rsm = apool.tile([64, 32], f16, tag="rsm", bufs=2, name="rsm")
            nc.sync.dma_start(
                out=rsm, in_=drow.rearrange("a (b c) -> (a b) c", b=16))
            rrm = apool.tile([64, 32], f32, tag="rrm", bufs=2, name="rrm")
            nc.vector.reciprocal(rrm, rsm)
            rc16 = apool.tile([64, 32], f16, tag="rc16", bufs=2, name="rc16")
            nc.vector.tensor_copy(rc16, rrm)
            scr = dpool.tile([64, 32], f16, tag="scr", bufs=2, name="scr")
            nc.sync.dma_start(out=scr, in_=rc16)
            rt = apool.tile([128, S], f16, tag="rt", bufs=2, name="rt")
            bc0 = bass.AP(tensor=scr.tensor, offset=scr.offset,
                          ap=[[0, DK], [1, S]])
            bc1 = bass.AP(tensor=scr.tensor, offset=scr.offset + S,
                          ap=[[0, DK], [1, S]])
            nc.sync.dma_start(out=rt[0:DK, :], in_=bc0)
            nc.gpsimd.dma_start(out=rt[DK:128, :], in_=bc1)
            nc.vector.tensor_mul(ao[m], ao[m], rt)

        # ---- main pipeline over m-tiles ----
        # Each (m, j) block: scores -> [128-mode filler PE work that hides
        # the exp window on ACT] -> attnV.  m=5 runs j=1 first so the last
        # (small) j=0 window lands next to the tail.
        q_proj(0)
        k_proj(0)
        for m in range(DT):
            drow = dpool.tile([HPM * NB, 512], f16, tag="drow", bufs=2,
                              name="drow")
            jorder = (1, 0) if m == 5 else (0, 1)
            for j in jorder:
                ets = scores(m, j)
                if j == 0 and m < 5:
                    if m == 0:
                        v_proj(0, 512, range(0, 4))
                    else:
                        q_proj(m + 1)
                elif j == 1 and m < 5:
                    if m == 0:
                        q_proj(1)
                        v_proj(0, 512, range(4, 8))
                        k_proj(1)
                    else:
                        k_proj(m + 1)
                    if m == 2:
                        v_proj(8, 256, range(0, 4))
                    elif m == 3:
                        v_proj(8, 256, range(4, 8))
                    elif m == 4:
                        for g in range(0, 4):
                            oproj_pass(g, range(4), True)
                elif m == 5 and j == 1:
                    for g in range(4, 10):
                        oproj_pass(g, range(4), True)
                elif m == 5 and j == 0:
                    for g in range(10, 12):
                        oproj_pass(g, range(4), True)
                attn_v(m, j, ets, drow)
            normalize(m, drow)
        for g in range(12):
            oproj_pass(g, (4, 5), False)

    nc.compile()
    return nc


def prepare(x, mask, W_q, b_q, W_k, b_k, W_v, b_v, W_o, b_o):
    """Compile (cached) and build per-core input maps."""
    x = np.asarray(x, np.float32)
    mask_b = np.asarray(mask).reshape(S, S) != 0
    blocks, patterns = _classify_mask(mask_b)
    key = mask_b.tobytes()
    if key not in _CACHE:
        pat_width = sum(2 * p.shape[1] for p in patterns)
        _CACHE[key] = (_build(blocks, patterns, pat_width), patterns)
    nc, patterns = _CACHE[key]

    xT = np.ascontiguousarray(x.transpose(0, 2, 1))          # [B, D, S]

    def swz(w):
        # [D, N] -> [128, DT, N]: partition-major
        w = np.asarray(w, np.float16)
        return np.ascontiguousarray(
            w.reshape(DT, 128, w.shape[1]).transpose(1, 0, 2))

    def swz_m(w):
        # [D, D] -> [128, m, k, 128] so per-m slices are contiguous
        w = np.asarray(w, np.float16)
        return np.ascontiguousarray(
            w.reshape(DT, 128, DT, 128).transpose(1, 2, 0, 3))

    base = {
        "wq": swz_m(np.asarray(W_q, np.float32) / np.sqrt(DK)),
        "wk": swz_m(W_k),
        "wv": swz(W_v),
        "wo": swz(W_o),
        "bq": np.ascontiguousarray(
            (np.asarray(b_q, np.float32) / np.sqrt(DK)).reshape(DT, 128).T),
        "bk": np.ascontiguousarray(
            np.asarray(b_k, np.float32).reshape(DT, 128).T),
        "bv": np.ascontiguousarray(np.broadcast_to(
            np.asarray(b_v, np.float32).reshape(1, H, DK), (128, H, DK))),
        "bo": np.ascontiguousarray(
            np.asarray(b_o, np.float32).reshape(DT, 128).T),
    }
    if patterns:
        base["masks"] = np.ascontiguousarray(
            np.concatenate([np.concatenate([p, p], axis=1)
                            for p in patterns], axis=1))
    in_maps = [dict(base, xT=swz(xT[c])) for c in range(B)]
    return nc, in_maps


def kernel(**inputs):
    from concourse.bass_utils import run_bass_kernel_spmd

    nc, in_maps = prepare(**inputs)
    res = run_bass_kernel_spmd(nc, in_maps, core_ids=list(range(B)))
    out = np.stack([res.results[c]["yT"].T.astype(np.float32)
                    for c in range(B)], axis=0)
    return np.ascontiguousarray(out)


# revision 12
# speedup vs baseline: 1.1609x; 1.0796x over previous
"""Multi-head attention (B=8, S=1024, D=768, H=12) on 8 TRN2 NeuronCores.

Sharding: batch-parallel — each core computes one batch item end-to-end
(weights replicated), so no collectives are needed. Host shards x, runs the
SPMD Bass kernel on cores 0-7, gathers per-core outputs.

Per-core dataflow ([feature, seq] transposed layout, fp16 matmuls, fp32 PSUM):
  qT = (W_q/8)^T x^T          [768, 1024]  partitions 0-63 = head 2m,
  kT = W_k^T x^T              [768, 1024]  64-127 = head 2m+1 per m-tile
  v  = x W_v (+ ones col)     [1024, 768]
  scores: per (m, j-block, k-tile i): TWO K=64 matmuls (head pair) into one
    PSUM tile — auto row-tiling (tile_position (0,0)/(64,0)) runs them
    concurrently in the 64x128-tiled PE array.
  exp: single gap-free ACT op per group covering both heads (ACT is the
    attention-phase bottleneck: (N+352)/1.2ns per op).
  attnV: per head, M=65 (64 v-dims + ones row -> softmax denom), N trimmed
    causally; evacuated as one [65,512] DVE copy, rows fanned out by DMA.
  normalize per m: denom rows -> DRAM, reload as [64,32] (all lanes),
    reciprocal, DRAM-bounce broadcast, one [128,1024] fp16 multiply.
  yT = W_o^T out^T: pass1 (k=0..3) interleaved into m=4 attention stalls,
    pass2 (k=4..5) after the last normalize; fp16 stores.
Host transposes/upcasts yT back to [1024, 768] fp32.
"""

import numpy as np

B, S, D, H, DK = 8, 1024, 768, 12, 64
DT = D // 128        # 6  d-model tiles
ST = S // 128        # 8  seq tiles
NB = S // 512        # 2  512-wide seq blocks
HPM = 2              # heads per 128-row m-tile

_CACHE = {}


def _classify_mask(mask_bool):
    """mask_bool: [S, S] (q, k). Per (j, i) block descriptors for the
    scoresT layout [k, q]: list over j (512-wide q blocks) of lists of
    (i, exp_lo, mul(lo,hi)|None, pat_id|None), plus deduped mask patterns
    (each [128, w] f16, [k, q], stored duplicated side by side so one DVE
    op can mask both heads' copies)."""
    patterns = []
    pat_index = {}
    blocks = []
    for j in range(NB):
        row = []
        for i in range(ST):
            sub = mask_bool[j * 512:(j + 1) * 512, i * 128:(i + 1) * 128].T
            # sub: [k 128, q 512]
            if not sub.any():
                continue
            col_any = sub.any(axis=0)
            col_all = sub.all(axis=0)
            exp_lo = int(np.argmax(col_any))
            assert not col_any[:exp_lo].any()
            mixed = ~col_all
            mixed[:exp_lo] = False
            mul = None
            pat_id = None
            if mixed.any():
                lo = int(np.argmax(mixed))
                hi = int(len(mixed) - np.argmax(mixed[::-1]))
                assert col_all[hi:].all() and col_all[exp_lo:lo].all()
                pat = sub[:, lo:hi].astype(np.float16)
                key = (pat.shape[1], pat.tobytes())
                if key not in pat_index:
                    pat_index[key] = len(patterns)
                    patterns.append(pat)
                mul = (lo, hi)
                pat_id = pat_index[key]
            row.append((i, exp_lo, mul, pat_id))
        blocks.append(row)
    return blocks, patterns


def _build(blocks, patterns, pat_width):
    import concourse.bass as bass
    import concourse.bacc as bacc
    import concourse.mybir as mybir
    import concourse.tile as tile
    from contextlib import ExitStack

    f32 = mybir.dt.float32
    f16 = mybir.dt.float16
    AF = mybir.ActivationFunctionType

    nc = bacc.Bacc("TRN2", target_bir_lowering=False, debug=False)

    xT_d = nc.dram_tensor("xT", [128, DT, S], f16, kind="ExternalInput").ap()
    wq_d = nc.dram_tensor("wq", [128, DT, DT, 128], f16,
                          kind="ExternalInput").ap()   # [p, m, k, c]
    wk_d = nc.dram_tensor("wk", [128, DT, DT, 128], f16,
                          kind="ExternalInput").ap()
    wv_d = nc.dram_tensor("wv", [128, DT, D], f16, kind="ExternalInput").ap()
    wo_d = nc.dram_tensor("wo", [128, DT, D], f16, kind="ExternalInput").ap()
    bq_d = nc.dram_tensor("bq", [128, DT], f32, kind="ExternalInput").ap()
    bk_d = nc.dram_tensor("bk", [128, DT], f32, kind="ExternalInput").ap()
    bv_d = nc.dram_tensor("bv", [128, H, DK], f32,
                          kind="ExternalInput").ap()
    bo_d = nc.dram_tensor("bo", [128, DT], f32, kind="ExternalInput").ap()
    yT_d = nc.dram_tensor("yT", [D, S], f16, kind="ExternalOutput").ap()
    if pat_width:
        mk_d = nc.dram_tensor("masks", [128, pat_width], f16,
                              kind="ExternalInput").ap()

    pat_off = []
    off = 0
    for p in patterns:
        pat_off.append(off)
        off += 2 * p.shape[1]

    with tile.TileContext(nc) as tc, ExitStack() as ctx:
        cpool = ctx.enter_context(tc.tile_pool(name="cpool", bufs=1))
        qT = cpool.tile([128, DT, S], f16)
        kT = cpool.tile([128, DT, S], f16)
        vE = cpool.tile([128, ST, H * 65 + 63], f16)
        ao = [cpool.tile([128, S], f16, name=f"ao{m}") for m in range(DT)]
        ya = [cpool.tile([128, 512], f16, name=f"ya{g}") for g in range(12)]
        bvb = cpool.tile([128, H, DK], f32)
        bqs = cpool.tile([128, DT], f32)
        bks = cpool.tile([128, DT], f32)
        bos = cpool.tile([128, DT], f32)

        xt = cpool.tile([128, DT, S], f16)
        wvt = cpool.tile([128, DT, D], f16)
        wqt = cpool.tile([128, DT, DT, 128], f16)
        wkt = cpool.tile([128, DT, DT, 128], f16)
        wot = cpool.tile([128, DT, D], f16)

        # staged input loads across both DMA rings. x chunks + wq0/wk0
        # gate the first matmuls; chains keep the SDMA round-robin from
        # starving the early pieces.
        mks = cpool.tile([128, pat_width], f16, name="mks") \
            if pat_width else None
        d_x0 = nc.gpsimd.dma_start(out=xt[:, 0:2], in_=xT_d[:, 0:2])
        d_x1 = nc.gpsimd.dma_start(out=xt[:, 2:4], in_=xT_d[:, 2:4])
        d_wva = nc.gpsimd.dma_start(out=wvt[:, :, 0:512],
                                    in_=wv_d[:, :, 0:512])
        d_wvb = nc.gpsimd.dma_start(out=wvt[:, :, 512:768],
                                    in_=wv_d[:, :, 512:768])
        d_wo = nc.gpsimd.dma_start(out=wot, in_=wo_d)
        gchain = [d_x0, d_x1, d_wva, d_wvb, d_wo]
        d_x2 = nc.sync.dma_start(out=xt[:, 4:6], in_=xT_d[:, 4:6])
        d_wq0 = nc.sync.dma_start(out=wqt[:, 0], in_=wq_d[:, 0])
        d_wk0 = nc.sync.dma_start(out=wkt[:, 0], in_=wk_d[:, 0])
        d_bq = nc.sync.dma_start(out=bqs, in_=bq_d)
        d_bk = nc.sync.dma_start(out=bks, in_=bk_d)
        schain = [d_x2, d_wq0, d_wk0, d_bq, d_bk]
        if pat_width:
            schain.append(nc.sync.dma_start(out=mks, in_=mk_d))
        schain.append(nc.sync.dma_start(out=bvb, in_=bv_d))
        schain.append(nc.sync.dma_start(out=wqt[:, 1:6], in_=wq_d[:, 1:6]))
        schain.append(nc.sync.dma_start(out=wkt[:, 1:6], in_=wk_d[:, 1:6]))
        schain.append(nc.sync.dma_start(out=bos, in_=bo_d))
        for chain in (gchain, schain):
            for a, b in zip(chain[1:], chain):
                tile.add_dep_helper(a.ins, b.ins, reason="stage inputs")

        # ones columns for the softmax-denominator rows; zero the tail so
        # the last heads' 128-col lhsT windows read defined data
        for st in range(ST):
            ve_h = vE[:, st, 0:H * 65].rearrange("p (h e) -> p h e", e=65)
            nc.vector.memset(ve_h[:, :, DK:DK + 1], 1.0)
        nc.vector.memset(vE[:, :, H * 65:], 0.0)

        ps_pj = ctx.enter_context(
            tc.tile_pool(name="ps_pj", bufs=2, space="PSUM"))
        ps_s = ctx.enter_context(
            tc.tile_pool(name="ps_s", bufs=2, space="PSUM"))
        ps_o = ctx.enter_context(
            tc.tile_pool(name="ps_o", bufs=2, space="PSUM"))
        apool = ctx.enter_context(tc.tile_pool(name="apool", bufs=1))
        dpool = ctx.enter_context(
            tc.tile_pool(name="dpool", bufs=1, space="DRAM"))

        def q_proj(m):
            for nb in range(NB):
                pq = ps_pj.tile([128, 512], f32, tag="pj", name="pq")
                for k in range(DT):
                    nc.tensor.matmul(
                        pq, wqt[:, m, k], xt[:, k, nb * 512:(nb + 1) * 512],
                        start=(k == 0), stop=(k == DT - 1))
                nc.vector.tensor_scalar_add(
                    qT[:, m, nb * 512:(nb + 1) * 512], pq, bqs[:, m:m + 1])

        def k_proj(m):
            for nb in range(NB):
                pk = ps_pj.tile([128, 512], f32, tag="pj", name="pk")
                for k in range(DT):
                    nc.tensor.matmul(
                        pk, wkt[:, m, k], xt[:, k, nb * 512:(nb + 1) * 512],
                        start=(k == 0), stop=(k == DT - 1))
                nc.vector.tensor_scalar_add(
                    kT[:, m, nb * 512:(nb + 1) * 512], pk, bks[:, m:m + 1])

        def v_proj(h0, w, sts):
            for st in sts:
                pv = ps_pj.tile([128, 512], f32, tag="pj", name="pv")
                for k in range(DT):
                    nc.tensor.matmul(
                        pv[:, :w],
                        xt[:, k, st * 128:(st + 1) * 128],
                        wvt[:, k, h0 * DK:h0 * DK + w],
                        start=(k == 0), stop=(k == DT - 1))
                nh = w // DK
                ve_h = vE[:, st, h0 * 65:(h0 + nh) * 65].rearrange(
                    "p (h e) -> p h e", e=65)
                nc.vector.tensor_add(
                    ve_h[:, :, 0:DK],
                    pv[:, :w].rearrange("p (h d) -> p h d", d=DK),
                    bvb[:, h0:h0 + nh, :])

        def oproj_pass(g, ks, first):
            mo, nb = g // NB, g % NB
            py = ps_pj.tile([128, 512], f32, tag="pj", name="py")
            for n, k in enumerate(ks):
                nc.tensor.matmul(
                    py, wot[:, k, mo * 128:(mo + 1) * 128],
                    ao[k][:, nb * 512:(nb + 1) * 512],
                    start=(n == 0), stop=(n == len(ks) - 1))
            if first:
                nc.vector.tensor_scalar_add(ya[g], py, bos[:, mo:mo + 1])
            else:
                yt = apool.tile([128, 512], f16, tag="yt", bufs=3, name="yt")
                nc.vector.tensor_add(yt, py, ya[g])
                eng = nc.sync if g % 2 == 0 else nc.gpsimd
                eng.dma_start(
                    out=yT_d[mo * 128:(mo + 1) * 128,
                             nb * 512:(nb + 1) * 512],
                    in_=yt)

        def scores(m, j):
            """Emit paired-head score matmuls + exp + mask for all groups
            of (m, j). Returns [(i, lo, w, et)] for attnV."""
            out = []
            for (i, lo, mul, pat_id) in blocks[j]:
                w = 512 - lo
                pss = ps_s.tile([128, 1024], f32, tag="ps", name="pss")
                et = apool.tile([128, 1024], f16, tag="et", bufs=10,
                                name="et")
                for hh in range(HPM):
                    p0, p1 = hh * 64, hh * 64 + 64
                    nc.tensor.matmul(
                        pss[:, hh * 512 + lo:(hh + 1) * 512],
                        kT[p0:p1, m, i * 128:(i + 1) * 128],
                        qT[p0:p1, m, j * 512 + lo:(j + 1) * 512],
                        start=True, stop=True)
                # one ACT op for both heads; strided AP skips the dead
                # [512-lo, 512) gap (matmul outs can't cross PSUM banks)
                p3 = pss.rearrange("p (two q) -> p two q", two=2)
                e3 = et.rearrange("p (two q) -> p two q", two=2)
                nc.scalar.activation(out=e3[:, :, lo:512],
                                     in_=p3[:, :, lo:512], func=AF.Exp)
                if mul is not None:
                    mlo, mhi = mul
                    wm = mhi - mlo
                    m3 = mks[:, pat_off[pat_id]:pat_off[pat_id] + 2 * wm
                             ].rearrange("p (two q) -> p two q", two=2)
                    nc.vector.tensor_mul(
                        e3[:, :, mlo:mhi], e3[:, :, mlo:mhi], m3)
                out.append((i, lo, w, et))
            return out

        def attn_v(m, j, ets, rsm):
            for hh in range(HPM):
                h = m * HPM + hh
                po = ps_o.tile([128, 512], f32, tag="po", name="po")
                for n, (i, lo, w, et) in enumerate(ets):
                    nc.tensor.matmul(
                        po[:, lo:512],
                        vE[:, i, h * 65:h * 65 + 128],
                        et[:, hh * 512 + lo:(hh + 1) * 512],
                        start=(n == 0), stop=(n == len(ets) - 1))
                stg = apool.tile([65, 512], f16, tag="stg", bufs=4,
                                 name="stg")
                nc.vector.tensor_copy(stg, po[0:65, :])
                nc.gpsimd.dma_start(
                    out=ao[m][hh * DK:(hh + 1) * DK,
                              j * 512:(j + 1) * 512],
                    in_=stg[0:DK, :])
                # denominator row straight into the [32,32] reciprocal
                # staging tile via partition-scatter DMA
                nc.sync.dma_start(out=rsm[hh * 16:(hh + 1) * 16, :],
                                  in_=stg[DK:65, :])

        def normalize(m, j, rsm):
            rrm = apool.tile([32, 32], f32, tag="rrm", bufs=2, name="rrm")
            nc.vector.reciprocal(rrm, rsm)
            rc16 = apool.tile([32, 32], f16, tag="rc16", bufs=2, name="rc16")
            nc.vector.tensor_copy(rc16, rrm)
            scr = dpool.tile([32, 32], f16, tag="scr", bufs=2, name="scr")
            nc.sync.dma_start(out=scr, in_=rc16)
            rt = apool.tile([128, 512], f16, tag="rt", bufs=2, name="rt")
            bc0 = bass.AP(tensor=scr.tensor, offset=scr.offset,
                          ap=[[0, DK], [1, 512]])
            bc1 = bass.AP(tensor=scr.tensor, offset=scr.offset + 512,
                          ap=[[0, DK], [1, 512]])
            nc.sync.dma_start(out=rt[0:DK, :], in_=bc0)
            nc.gpsimd.dma_start(out=rt[DK:128, :], in_=bc1)
            cols = slice(j * 512, (j + 1) * 512)
            nc.vector.tensor_mul(ao[m][:, cols], ao[m][:, cols], rt)

        # ---- main pipeline over m-tiles ----
        # Each (m, j) block: scores -> [128-mode filler PE work that hides
        # the exp window on ACT] -> attnV.  m=5 runs j=1 first so the last
        # (small) j=0 window lands next to the tail.
        q_proj(0)
        k_proj(0)
        for m in range(DT):
            jorder = (1, 0) if m == 5 else (0, 1)
            for j in jorder:
                rsm = apool.tile([32, 32], f16, tag="rsm", bufs=2,
                                 name="rsm")
                ets = scores(m, j)
                if j == 0 and m < 5:
                    if m == 0:
                        v_proj(0, 512, range(0, 4))
                    else:
                        q_proj(m + 1)
                elif j == 1 and m < 5:
                    if m == 0:
                        q_proj(1)
                        v_proj(0, 512, range(4, 8))
                        k_proj(1)
                    else:
                        k_proj(m + 1)
                    if m == 2:
                        v_proj(8, 256, range(0, 4))
                    elif m == 3:
                        v_proj(8, 256, range(4, 8))
                    elif m == 4:
                        for g in range(0, 4):
                            oproj_pass(g, range(4), True)
                elif m == 5 and j == 1:
                    for g in range(4, 10):
                        oproj_pass(g, range(4), True)
                elif m == 5 and j == 0:
                    for g in range(10, 12):
                        oproj_pass(g, range(4), True)
                    # nb=1 output halves: every ao column they read is
                    # normalized once norm(5, j=1) lands -> tail work
                    # pulled into this window
                    for g in range(1, 12, 2):
                        oproj_pass(g, (4, 5), False)
                attn_v(m, j, ets, rsm)
                normalize(m, j, rsm)
        for g in range(0, 12, 2):
            oproj_pass(g, (4, 5), False)

    nc.compile()
    return nc


def prepare(x, mask, W_q, b_q, W_k, b_k, W_v, b_v, W_o, b_o):
    """Compile (cached) and build per-core input maps."""
    x = np.asarray(x, np.float32)
    mask_b = np.asarray(mask).reshape(S, S) != 0
    blocks, patterns = _classify_mask(mask_b)
    key = mask_b.tobytes()
    if key not in _CACHE:
        pat_width = sum(2 * p.shape[1] for p in patterns)
        _CACHE[key] = (_build(blocks, patterns, pat_width), patterns)
    nc, patterns = _CACHE[key]

    xT = np.ascontiguousarray(x.transpose(0, 2, 1))          # [B, D, S]

    def swz(w):
        # [D, N] -> [128, DT, N]: partition-major
        w = np.asarray(w, np.float16)
        return np.ascontiguousarray(
            w.reshape(DT, 128, w.shape[1]).transpose(1, 0, 2))

    def swz_m(w):
        # [D, D] -> [128, m, k, 128] so per-m slices are contiguous
        w = np.asarray(w, np.float16)
        return np.ascontiguousarray(
            w.reshape(DT, 128, DT, 128).transpose(1, 2, 0, 3))

    base = {
        "wq": swz_m(np.asarray(W_q, np.float32) / np.sqrt(DK)),
        "wk": swz_m(W_k),
        "wv": swz(W_v),
        "wo": swz(W_o),
        "bq": np.ascontiguousarray(
            (np.asarray(b_q, np.float32) / np.sqrt(DK)).reshape(DT, 128).T),
        "bk": np.ascontiguousarray(
            np.asarray(b_k, np.float32).reshape(DT, 128).T),
        "bv": np.ascontiguousarray(np.broadcast_to(
            np.asarray(b_v, np.float32).reshape(1, H, DK), (128, H, DK))),
        "bo": np.ascontiguousarray(
            np.asarray(b_o, np.float32).reshape(DT, 128).T),
    }
    if patterns:
        base["masks"] = np.ascontiguousarray(
            np.concatenate([np.concatenate([p, p], axis=1)
                            for p in patterns], axis=1))
    in_maps = [dict(base, xT=swz(xT[c])) for c in range(B)]
    return nc, in_maps


def kernel(**inputs):
    from concourse.bass_utils import run_bass_kernel_spmd

    nc, in_maps = prepare(**inputs)
    res = run_bass_kernel_spmd(nc, in_maps, core_ids=list(range(B)))
    out = np.stack([res.results[c]["yT"].T.astype(np.float32)
                    for c in range(B)], axis=0)
    return np.ascontiguousarray(out)
